# revision 19
# baseline (speedup 1.0000x reference)
"""Trainium2 Bass kernel for nn_Logic_Model_80607946211458.

Strategy
--------
The model is a tiny batch-elementwise computation over B=500 event rows
(30 body-predicate times each) plus O(1) bookkeeping on the (2,32) rule
tensor A.  It is overhead/memory-regime, so the kernel minimizes device
instruction and DMA count:

* Host (inside ``kernel()``): the A top-k bookkeeping — top-3 indices,
  pair validity, gathered pair columns and the piecewise-constant values
  of the relaxed temporal-relation softmax (which depend only on
  ``prob``) — is computed on the host and baked into the compiled kernel
  as immediates / packed constant columns.  This only depends on the
  tiny inputs (A, pi, prob, base, formula_weight).
* Device: 8-way data-parallel over the batch (63 rows per NeuronCore,
  batch on SBUF partitions).  One packed [63, 149] f32 DMA in, ~30
  fused DVE/ACT instructions (straight-line, raw Bass with manual
  semaphores — no Tile tail barriers), one [63, 3] DMA out.
"""

import sys

import numpy as np

if "/opt/trn_rl_repo" not in sys.path:
    sys.path.insert(0, "/opt/trn_rl_repo")

import concourse.bass as bass
import concourse.mybir as mybir
from concourse.bass_utils import run_bass_kernel_spmd


def _ensure_axon_hooks():
    """Provide ``antenv.axon_hooks`` if the image lacks it.

    ``run_bass_kernel_spmd(trace=True)`` (or BASS_TRACE=1) imports
    ``antenv.axon_hooks`` unconditionally; some images ship ``antenv``
    without it.  Register an in-memory module exposing the documented
    get/set API, lazily wiring the ctypes NTFF hook from
    ``trn_agent_boot`` when available (else tracing degrades gracefully).
    """
    try:
        import antenv.axon_hooks  # noqa: F401
        return
    except ImportError:
        pass
    try:
        import antenv
    except ImportError:
        return
    import types

    mod = types.ModuleType("antenv.axon_hooks")
    holder = {"hook": None, "tried": False}

    def set_axon_ntff_profile_hook(h):
        holder["hook"] = h
        holder["tried"] = True

    def get_axon_ntff_profile_hook():
        if holder["hook"] is None and not holder["tried"]:
            holder["tried"] = True
            try:
                from trn_agent_boot.trn_boot import _ntff_profile_via_ctypes
                holder["hook"] = _ntff_profile_via_ctypes(
                    "/opt/axon/libaxon_pjrt.so")
            except Exception:
                holder["hook"] = None
        return holder["hook"]

    mod.set_axon_ntff_profile_hook = set_axon_ntff_profile_hook
    mod.get_axon_ntff_profile_hook = get_axon_ntff_profile_hook
    sys.modules["antenv.axon_hooks"] = mod
    antenv.axon_hooks = mod


_ensure_axon_hooks()

NCORES = 8
NB = 30          # body predicates
KSEL = 3         # top-k predicates per formula
SIGMA = 0.1
TEMP = 0.07
TOL = 0.02
_PA = np.array([0, 0, 1])
_PB = np.array([1, 2, 2])

# ---- packed input column layout (all float32) ----
C_DS2 = 0            # 60: data_sample duplicated twice
C_DSP = 60           # 6:  data_sample[:, p_c] for both formulas
C_DSQ = 66           # 6:  data_sample[:, q_c]
C_T = 72             # 1:  head event time t
C_ABC = 73           # 60: A[0,:30] | A[1,:30], broadcast down rows
C_EC = 133           # 2:  A[i,30]+A[i,31]-K  per formula
C_LP = 135           # 2:  log(pi[1:])
C_FW = 137           # 2:  formula_weight
C_MSK = 139          # 6:  pair-validity mask
C_SEL = 145          # 2:  1 if formula has >=1 valid pair else 0
C_ADD = 147          # 2:  1 - sel
NCOL = 149

F32 = mybir.dt.float32
ALU = mybir.AluOpType
ACTF = mybir.ActivationFunctionType

# build cache: cfg-tuple -> (nc, keepalive_exitstack)
_BUILD_CACHE: dict = {}
LAST_RESULT = None  # BassKernelResults of the most recent run (for test harness)


def _rrf_region_value(j: int, prob: np.ndarray) -> float:
    """rrf value when td falls in region j (0: >TOL, 1: |td|<TOL, 2: <-TOL,
    -1: exactly on a boundary).  Mirrors reference's custom_softmax of
    tbi*prob elementwise, computed in float64."""
    p = prob.astype(np.float64)
    c = np.zeros(3, np.float64)
    if j >= 0:
        c[j] = 1.0
    c3 = 1.0 - p[0] * c[0] - p[1] * c[1] - p[2] * c[2]
    tbi = np.array([c[0], c[1], c[2], c3], np.float64)
    u = tbi * p
    w = np.exp(u / TEMP)
    return float((w * u).sum() / w.sum())


def _f32(x) -> float:
    """Round a python/numpy scalar to float32 and return as python float."""
    return float(np.float32(x))


def _build(cfg):
    """Build + finalize the Bass module for one core (SPMD; all cores run it).

    cfg is a tuple of hashable scalars:
      (P, r1, dr0, dr2, need_boundary, drb, need_mask, need_sel,
       neg_inv_sigma, neg_inv_temp, b0, lp0c)
    """
    (P, r1, dr0, dr2, need_boundary, drb, need_mask, need_sel,
     neg_inv_sigma, neg_inv_temp, b0, lp0c) = cfg

    from contextlib import ExitStack

    ctx = ExitStack()
    nc = bass.Bass()
    xd = nc.dram_tensor("x", [P, NCOL], F32, kind="ExternalInput")
    od = nc.dram_tensor("o", [P, 3], F32, kind="ExternalOutput")

    sb = lambda name, shape: ctx.enter_context(nc.sbuf_tensor(name, shape, F32))
    sem = lambda name: ctx.enter_context(nc.semaphore(name))

    X = sb("xt", [P, NCOL])
    q01 = sb("q01", [P, 2 * NB])
    mm = sb("mm", [P, 2 * NB])
    D = sb("dots", [P, 2])
    Mb = sb("mbt", [P, 2])
    dsh = sb("dsh", [P, 2])
    ab = sb("ab", [P, 2])
    rrf2 = sb("rrf2", [P, 6]) if need_boundary else None
    rrf3 = sb("rrf3", [P, 6]) if need_boundary else None
    col2 = sb("col2", [P, 2]) if need_sel else None
    col3 = sb("col3", [P, 2]) if need_sel else None
    feat = sb("feat", [P, 2])
    sigm = sb("sigm", [P, 2])
    td = sb("td", [P, 6])
    s0 = sb("s0", [P, 6])
    s2 = sb("s2", [P, 6])
    rrf = sb("rrf", [P, 6])
    em = sb("em", [P, 6])
    emm = sb("emm", [P, 6]) if need_mask else em
    erm = sb("erm", [P, 6])
    den = sb("den", [P, 2])
    num = sb("num", [P, 2])
    lden = sb("lden", [P, 2])
    rden = sb("rden", [P, 2])
    col = sb("col", [P, 2])
    x1 = sb("x1", [P, 2])
    x2 = sb("x2", [P, 2])
    sg = sb("sg", [P, 2])
    cur = sb("cur", [P, 2])
    lcur = sb("lcur", [P, 2])
    tcn = sb("tcn", [P, 2])
    h2 = sb("h2", [P, 2])
    term = sb("term", [P, 2])
    r1t = sb("r1t", [P, 2])
    r2t = sb("r2t", [P, 2])
    O = sb("ot", [P, 3])
    if need_boundary:
        sbp = sb("sbp", [P, 6])
        sbn = sb("sbn", [P, 6])

    rrf_f = rrf3 if need_boundary else rrf

    dma_in = sem("dma_in")
    dma_out = sem("dma_out")
    v1 = sem("v1")
    v1b = sem("v1b")
    v2 = sem("v2")
    a1 = sem("a1")
    a1b = sem("a1b")
    a2 = sem("a2")
    cdone = sem("cdone")

    tS = X[:, C_T:C_T + 1]  # per-partition scalar t

    with nc.Block() as block:

        @block.sync
        def _(sync):
            sync.dma_start(out=X[:], in_=xd[:]).then_inc(dma_in, 16)
            sync.wait_ge(cdone, 2)
            sync.dma_start(out=od[:], in_=O[:]).then_inc(dma_out, 16)
            sync.wait_ge(dma_out, 16)

        # Engines are deep pipelines with NO interlock between an
        # instruction's writeback and the next instruction's operand read —
        # even on the SAME engine (CoreSim race detector + observed HW
        # staleness).  Every RAW edge is covered by a drain() at a
        # dependency-layer boundary; cross-engine signals use
        # drain().then_inc(sem).

        @block.vector
        def _(vector):
            v = nc.vector
            v.wait_ge(dma_in, 16)
            # L1: ind = (ds <= t); q_i = ind * A_i; dot_i = sum(q_i); td
            v.scalar_tensor_tensor(
                out=q01[:, 0:NB], in0=X[:, C_DS2:C_DS2 + NB], scalar=tS,
                in1=X[:, C_ABC:C_ABC + NB],
                op0=ALU.is_le, op1=ALU.mult, accum_out=D[:, 0:1])
            v.scalar_tensor_tensor(
                out=q01[:, NB:2 * NB], in0=X[:, C_DS2 + NB:C_DS2 + 2 * NB],
                scalar=tS, in1=X[:, C_ABC + NB:C_ABC + 2 * NB],
                op0=ALU.is_le, op1=ALU.mult, accum_out=D[:, 1:2])
            v.tensor_sub(out=td[:], in0=X[:, C_DSP:C_DSP + 6],
                         in1=X[:, C_DSQ:C_DSQ + 6])
            v.drain()
            # L2: mbt products; dot shift; td region indicators
            v.tensor_mul(out=mm[:], in0=q01[:], in1=X[:, C_DS2:C_DS2 + 2 * NB])
            v.tensor_add(out=dsh[:], in0=D[:], in1=X[:, C_EC:C_EC + 2])
            v.tensor_scalar(out=s0[:], in0=td[:], scalar1=_f32(TOL),
                            scalar2=dr0, op0=ALU.is_gt, op1=ALU.mult)
            v.tensor_scalar(out=s2[:], in0=td[:], scalar1=_f32(-TOL),
                            scalar2=dr2, op0=ALU.is_lt, op1=ALU.mult)
            if need_boundary:
                v.tensor_scalar(out=sbp[:], in0=td[:], scalar1=_f32(TOL),
                                scalar2=drb, op0=ALU.is_equal, op1=ALU.mult)
                v.tensor_scalar(out=sbn[:], in0=td[:], scalar1=_f32(-TOL),
                                scalar2=drb, op0=ALU.is_equal, op1=ALU.mult)
            v.drain()
            # L3: mbt reduce; rrf assembly
            v.tensor_reduce(
                out=Mb[:], in_=mm[:].rearrange("p (f j) -> p f j", j=NB),
                axis=mybir.AxisListType.X, op=ALU.max)
            v.scalar_tensor_tensor(
                out=rrf[:], in0=s0[:], scalar=r1, in1=s2[:],
                op0=ALU.add, op1=ALU.add)
            if need_boundary:
                v.drain()
                v.tensor_add(out=rrf2[:], in0=rrf[:], in1=sbp[:])
                v.drain()
                v.tensor_add(out=rrf3[:], in0=rrf2[:], in1=sbn[:])
            v.drain().then_inc(v1, 1)

            # ---- after ACT computed feat/sigm/em ----
            v.wait_ge(a1, 1)
            if need_mask:
                v.tensor_mul(out=emm[:], in0=em[:], in1=X[:, C_MSK:C_MSK + 6])
                v.drain()
            v.tensor_mul(out=erm[:], in0=emm[:], in1=rrf_f[:])
            v.tensor_reduce(
                out=den[:], in_=emm[:].rearrange("p (f k) -> p f k", k=3),
                axis=mybir.AxisListType.X, op=ALU.add)
            v.drain()
            v.tensor_reduce(
                out=num[:], in_=erm[:].rearrange("p (f k) -> p f k", k=3),
                axis=mybir.AxisListType.X, op=ALU.add)
            v.drain().then_inc(v1b, 1)
            # 1/den comes back from ACT as Exp(-Ln(den)) — InstReciprocal's
            # writeback is fully asynchronous (stale reads observed on HW),
            # so it cannot be used at all.
            v.wait_ge(a1b, 1)
            v.tensor_mul(out=col[:], in0=num[:], in1=rden[:])
            v.drain()
            col_f = col
            if need_sel:
                v.tensor_mul(out=col2[:], in0=col[:], in1=X[:, C_SEL:C_SEL + 2])
                v.drain()
                v.tensor_add(out=col3[:], in0=col2[:], in1=X[:, C_ADD:C_ADD + 2])
                v.drain()
                col_f = col3
            v.tensor_mul(out=x1[:], in0=feat[:], in1=col_f[:])
            v.drain()
            v.tensor_mul(out=x2[:], in0=x1[:], in1=X[:, C_FW:C_FW + 2])
            v.drain()
            v.tensor_mul(out=sg[:], in0=sigm[:], in1=x2[:])
            v.drain()
            v.tensor_scalar_add(out=cur[:], in0=sg[:], scalar1=b0)
            v.tensor_mul(out=h2[:], in0=sigm[:], in1=sg[:])
            v.drain().then_inc(v2, 1)
            v.tensor_scalar(out=tcn[:], in0=cur[:], scalar1=tS, scalar2=-1.0,
                            op0=ALU.mult, op1=ALU.mult)
            v.tensor_mul(out=term[:], in0=h2[:], in1=Mb[:])
            v.drain()
            # ---- after ACT computed lcur ----
            v.wait_ge(a2, 1)
            v.tensor_add(out=r1t[:], in0=lcur[:], in1=tcn[:])
            v.drain()
            v.tensor_add(out=r2t[:], in0=r1t[:], in1=term[:])
            v.drain()
            v.tensor_add(out=O[:, 1:3], in0=r2t[:], in1=X[:, C_LP:C_LP + 2])
            v.drain().then_inc(cdone, 1)

        @block.scalar
        def _(scalar):
            s = nc.scalar
            s.wait_ge(dma_in, 16)
            # log p*(z=0) column: lp0c - base*t
            s.activation(O[:, 0:1], tS, ACTF.Copy, bias=lp0c, scale=-b0)
            s.drain().then_inc(cdone, 1)
            s.wait_ge(v1, 1)
            s.activation(ab[:], dsh[:], ACTF.Abs)
            s.drain()
            s.activation(feat[:], ab[:], ACTF.Exp, scale=neg_inv_sigma)
            s.activation(sigm[:], Mb[:], ACTF.Sigmoid, bias=tS, scale=-1.0)
            s.activation(em[:], rrf_f[:], ACTF.Exp, scale=neg_inv_temp)
            s.drain().then_inc(a1, 1)
            s.wait_ge(v1b, 1)
            s.activation(lden[:], den[:], ACTF.Ln)
            s.drain()
            s.activation(rden[:], lden[:], ACTF.Exp, scale=-1.0)
            s.drain().then_inc(a1b, 1)
            s.wait_ge(v2, 1)
            s.activation(lcur[:], cur[:], ACTF.Ln)
            s.drain().then_inc(a2, 1)

    nc.finalize()
    return nc, ctx


def _prepare(t, data_sample, pi, A, base, formula_weight, prob):
    """Host-side bookkeeping + packed per-core inputs.  Returns (cfg, X)
    where X is [NCORES, P, NCOL] float32."""
    t = np.asarray(t, np.float32)
    ds = np.asarray(data_sample, np.float32)
    pi = np.asarray(pi, np.float32)
    A = np.asarray(A, np.float32)
    base = np.asarray(base, np.float32)
    fw = np.asarray(formula_weight, np.float32)
    prob = np.asarray(prob, np.float32)

    B = t.shape[0]
    P = -(-B // NCORES)  # rows per core (ceil)
    nF = A.shape[0]
    assert nF == 2 and ds.shape[1] == NB and A.shape[1] == NB + 2

    # --- A top-k bookkeeping (replicated, tiny) ---
    p_all = np.zeros(6, np.int64)
    q_all = np.zeros(6, np.int64)
    pv = np.zeros(6, np.float32)
    sel = np.zeros(2, np.float32)
    for i in range(nF):
        # top-3 by value desc, ties -> lower index first (lax.top_k semantics)
        idx = np.argsort(-A[i], kind="stable")[:KSEL]
        idx = np.sort(idx)
        valid = idx < NB
        pvi = (valid[_PA] & valid[_PB]).astype(np.float32)
        pv[3 * i:3 * i + 3] = pvi
        p_all[3 * i:3 * i + 3] = np.minimum(idx[_PA], NB - 1)
        q_all[3 * i:3 * i + 3] = np.minimum(idx[_PB], NB - 1)
        sel[i] = 1.0 if pvi.sum() > 0 else 0.0

    need_sel = bool((sel == 0.0).any())
    if need_sel:
        # keep den>0 so col is finite junk before the select overrides it
        for i in range(nF):
            if sel[i] == 0.0:
                pv[3 * i] = 1.0
    need_mask = bool((pv == 0.0).any())

    # --- piecewise-constant temporal-relation softmax values ---
    R0 = _rrf_region_value(0, prob)
    R1 = _rrf_region_value(1, prob)
    R2 = _rrf_region_value(2, prob)
    Rb = _rrf_region_value(-1, prob)

    dsP = ds[:, p_all]
    dsQ = ds[:, q_all]
    td_host = dsP - dsQ  # exactly what the device computes in f32
    need_boundary = bool((np.abs(td_host) == np.float32(TOL)).any())

    b0 = float(base[0])
    lp0c = _f32(np.float32(np.log(base[0])) + np.float32(np.log(pi[0])))

    cfg = (
        int(P), _f32(R1), _f32(R0 - np.float32(R1)), _f32(R2 - np.float32(R1)),
        need_boundary, _f32(Rb - np.float32(R1)), need_mask, need_sel,
        _f32(-1.0 / SIGMA), _f32(-1.0 / TEMP), _f32(b0), lp0c,
    )

    # --- pack per-core inputs ---
    BP = NCORES * P
    Xf = np.empty((BP, NCOL), np.float32)
    # benign padding rows (t=1, ds=0.5) keep all math finite
    ds_p = np.full((BP, NB), 0.5, np.float32)
    ds_p[:B] = ds
    t_p = np.ones((BP, 1), np.float32)
    t_p[:B] = t
    Xf[:, C_DS2:C_DS2 + NB] = ds_p
    Xf[:, C_DS2 + NB:C_DS2 + 2 * NB] = ds_p
    Xf[:, C_DSP:C_DSP + 6] = ds_p[:, p_all]
    Xf[:, C_DSQ:C_DSQ + 6] = ds_p[:, q_all]
    Xf[:, C_T:C_T + 1] = t_p
    arow = np.concatenate([A[0, :NB], A[1, :NB]])
    Xf[:, C_ABC:C_ABC + 2 * NB] = arow[None, :]
    ec = np.array([A[i, NB] + A[i, NB + 1] for i in range(nF)], np.float32) \
        - np.float32(KSEL)
    Xf[:, C_EC:C_EC + 2] = ec[None, :]
    Xf[:, C_LP:C_LP + 2] = np.log(pi[1:])[None, :]
    Xf[:, C_FW:C_FW + 2] = fw[None, :]
    Xf[:, C_MSK:C_MSK + 6] = pv[None, :]
    Xf[:, C_SEL:C_SEL + 2] = sel[None, :]
    Xf[:, C_ADD:C_ADD + 2] = (1.0 - sel)[None, :]

    return cfg, Xf.reshape(NCORES, P, NCOL)


def kernel(t, data_sample, pi, A, base, formula_weight, prob):
    global LAST_RESULT
    cfg, X = _prepare(t, data_sample, pi, A, base, formula_weight, prob)
    B = np.asarray(t).shape[0]
    P = cfg[0]

    cached = _BUILD_CACHE.get(cfg)
    if cached is None:
        cached = _build(cfg)
        _BUILD_CACHE[cfg] = cached
    nc, _ctx = cached

    in_maps = [{"x": np.ascontiguousarray(X[c])} for c in range(NCORES)]
    res = run_bass_kernel_spmd(nc, in_maps, core_ids=list(range(NCORES)))
    LAST_RESULT = res
    out = np.concatenate([res.results[c]["o"] for c in range(NCORES)], axis=0)
    return np.ascontiguousarray(out[:B]).astype(np.float32)


# revision 31
# speedup vs baseline: 1.1439x; 1.1439x over previous
"""Trainium2 Bass kernel for nn_Logic_Model_80607946211458.

Strategy
--------
The model is a tiny batch-elementwise computation over B=500 event rows
(30 body-predicate times each) plus O(1) bookkeeping on the (2,32) rule
tensor A.  It is overhead/memory-regime, so the kernel minimizes device
instruction and DMA count:

* Host (inside ``kernel()``): the A top-k bookkeeping — top-3 indices,
  pair validity, gathered pair columns and the piecewise-constant values
  of the relaxed temporal-relation softmax (which depend only on
  ``prob``) — is computed on the host and baked into the compiled kernel
  as immediates / packed constant columns.  This only depends on the
  tiny inputs (A, pi, prob, base, formula_weight).
* Device: 8-way data-parallel over the batch (63 rows per NeuronCore,
  batch on SBUF partitions).  One packed [63, 149] f32 DMA in, ~30
  fused DVE/ACT instructions (straight-line, raw Bass with manual
  semaphores — no Tile tail barriers), one [63, 3] DMA out.
"""

import sys

import numpy as np

if "/opt/trn_rl_repo" not in sys.path:
    sys.path.insert(0, "/opt/trn_rl_repo")

import concourse.bass as bass
import concourse.mybir as mybir
from concourse.bass_utils import run_bass_kernel_spmd


def _ensure_axon_hooks():
    """Provide ``antenv.axon_hooks`` if the image lacks it.

    ``run_bass_kernel_spmd(trace=True)`` (or BASS_TRACE=1) imports
    ``antenv.axon_hooks`` unconditionally; some images ship ``antenv``
    without it.  Register an in-memory module exposing the documented
    get/set API, lazily wiring the ctypes NTFF hook from
    ``trn_agent_boot`` when available (else tracing degrades gracefully).
    """
    try:
        import antenv.axon_hooks  # noqa: F401
        return
    except ImportError:
        pass
    try:
        import antenv
    except ImportError:
        return
    import types

    mod = types.ModuleType("antenv.axon_hooks")
    holder = {"hook": None, "tried": False}

    def set_axon_ntff_profile_hook(h):
        holder["hook"] = h
        holder["tried"] = True

    def get_axon_ntff_profile_hook():
        if holder["hook"] is None and not holder["tried"]:
            holder["tried"] = True
            try:
                from trn_agent_boot.trn_boot import _ntff_profile_via_ctypes
                holder["hook"] = _ntff_profile_via_ctypes(
                    "/opt/axon/libaxon_pjrt.so")
            except Exception:
                holder["hook"] = None
        return holder["hook"]

    mod.set_axon_ntff_profile_hook = set_axon_ntff_profile_hook
    mod.get_axon_ntff_profile_hook = get_axon_ntff_profile_hook
    sys.modules["antenv.axon_hooks"] = mod
    antenv.axon_hooks = mod


_ensure_axon_hooks()

NCORES = 8
NB = 30          # body predicates
KSEL = 3         # top-k predicates per formula
SIGMA = 0.1
TEMP = 0.07
TOL = 0.02
_PA = np.array([0, 0, 1])
_PB = np.array([1, 2, 2])

# ---- packed input column layout (all float32) ----
C_DS2 = 0            # 60: data_sample duplicated twice
C_DSP = 60           # 6:  data_sample[:, p_c] for both formulas
C_DSQ = 66           # 6:  data_sample[:, q_c]
C_T = 72             # 1:  head event time t
C_ABC = 73           # 60: A[0,:30] | A[1,:30], broadcast down rows
C_EC = 133           # 2:  A[i,30]+A[i,31]-K  per formula
C_FWT = 135          # 2:  formula_weight * (-TEMP)
C_PI = 137           # 2:  pi[1:]
C_NT = 139           # 1:  -t
C_M1 = 140           # 1:  -1.0
C_MG = 141           # 4:  int32 0x7EF127EA as float bits (reciprocal seed)
C_MSK = 145          # 6:  pair-validity mask
C_SEL = 151          # 2:  1 if formula has >=1 valid pair else 0
C_ADD = 153          # 2:  (1 - sel) * (-1/TEMP)
NCOL = 155

F32 = mybir.dt.float32
I32 = mybir.dt.int32
ALU = mybir.AluOpType
ACTF = mybir.ActivationFunctionType
MAGIC = 0x7EF127EA

# build cache: cfg-tuple -> (nc, keepalive_exitstack)
_BUILD_CACHE: dict = {}
LAST_RESULT = None  # BassKernelResults of the most recent run (for test harness)


def _rrf_region_value(j: int, prob: np.ndarray) -> float:
    """rrf value when td falls in region j (0: >TOL, 1: |td|<TOL, 2: <-TOL,
    -1: exactly on a boundary).  Mirrors reference's custom_softmax of
    tbi*prob elementwise, computed in float64."""
    p = prob.astype(np.float64)
    c = np.zeros(3, np.float64)
    if j >= 0:
        c[j] = 1.0
    c3 = 1.0 - p[0] * c[0] - p[1] * c[1] - p[2] * c[2]
    tbi = np.array([c[0], c[1], c[2], c3], np.float64)
    u = tbi * p
    w = np.exp(u / TEMP)
    return float((w * u).sum() / w.sum())


def _f32(x) -> float:
    """Round a python/numpy scalar to float32 and return as python float."""
    return float(np.float32(x))


def _build(cfg):
    """Build + finalize the Bass module for one core (SPMD; all cores run it).

    The temporal-relation softmax values are piecewise-constant in td and
    pre-scaled by -1/TEMP on the host (r1T/dr0T/dr2T/drbT), so the device
    computes rrfT = -rrf/TEMP directly; em = exp(rrfT).  The -TEMP
    un-scaling of col = num/den is folded into fw (C_FWT).  1/x is a
    2-step Newton iteration seeded by the classic exponent-flip bit trick
    (magic constant packed as int32 data) — the hardware InstReciprocal
    writeback is asynchronous and unusable.  ACT uses exactly two
    activation tables (Exp, Ln), each preloaded by a dummy op while
    DMA / DVE work is in flight.
    """
    (P, r1T, dr0T, dr2T, need_boundary, drbT, need_mask, need_sel,
     neg_inv_sigma, b0, lp0c) = cfg

    from contextlib import ExitStack

    ctx = ExitStack()
    nc = bass.Bass()
    xd = nc.dram_tensor("x", [P, NCOL], F32, kind="ExternalInput")
    od = nc.dram_tensor("o", [P, 3], F32, kind="ExternalOutput")

    sb = lambda name, shape: ctx.enter_context(nc.sbuf_tensor(name, shape, F32))
    sem = lambda name: ctx.enter_context(nc.semaphore(name))

    X = sb("xt", [P, NCOL])
    q01 = sb("q01", [P, 2 * NB])
    mm = sb("mm", [P, 2 * NB])
    D = sb("dots", [P, 2])
    Mb = sb("mbt", [P, 2])
    dsh = sb("dsh", [P, 2])
    ab = sb("ab", [P, 2])
    feat = sb("feat", [P, 2])
    featFW = sb("featFW", [P, 2])
    td = sb("td", [P, 6])
    s0 = sb("s0", [P, 6])
    s2 = sb("s2", [P, 6])
    rrf = sb("rrf", [P, 6])
    em = sb("em", [P, 6])
    e1 = sb("e1", [P, 2])
    emm = sb("emm", [P, 6]) if need_mask else em
    erm = sb("erm", [P, 6])
    Q = sb("q4", [P, 4])
    Y0 = sb("y0", [P, 4])
    Y1 = sb("y1", [P, 4])
    Y2 = sb("y2", [P, 4])
    T1 = sb("t1", [P, 4])
    T2 = sb("t2", [P, 4])
    T1b = sb("t1b", [P, 4])
    T2b = sb("t2b", [P, 4])
    num = sb("num", [P, 2])
    colT = sb("colT", [P, 2])
    col2 = sb("col2", [P, 2]) if need_sel else None
    col3 = sb("col3", [P, 2]) if need_sel else None
    fs2 = sb("fs2", [P, 2])
    sg = sb("sg", [P, 2])
    sm = sb("sm", [P, 2])
    cur = sb("cur", [P, 2])
    cur2 = sb("cur2", [P, 2])
    lcur = sb("lcur", [P, 2])
    tcn = sb("tcn", [P, 2])
    term = sb("term", [P, 2])
    ttx = sb("ttx", [P, 2])
    O = sb("ot", [P, 3])
    de_o = sb("de_o", [P, 1])
    dl_o = sb("dl_o", [P, 1])
    # initialized (preamble memset + barrier) constant for dummy table loads
    dum_in = nc.const_aps.aps[(F32, 1.0)].tensor[0:P, 0:1]
    if need_boundary:
        sbp = sb("sbp", [P, 6])
        sbn = sb("sbn", [P, 6])
        rrf2 = sb("rrf2", [P, 6])
        rrf3 = sb("rrf3", [P, 6])
    rrf_f = rrf3 if need_boundary else rrf

    dma_in = sem("dma_in")
    dma_out = sem("dma_out")
    v1 = sem("v1")
    v2 = sem("v2")
    a1 = sem("a1")
    a2 = sem("a2")
    cdone = sem("cdone")

    tS = X[:, C_T:C_T + 1]   # per-partition scalar t

    with nc.Block() as block:

        @block.sync
        def _(sync):
            sync.dma_start(out=X[:], in_=xd[:]).then_inc(dma_in, 16)
            sync.wait_ge(cdone, 1)
            sync.dma_start(out=od[:], in_=O[:]).then_inc(dma_out, 16)
            sync.wait_ge(dma_out, 16)

        @block.vector
        def _(vector):
            v = nc.vector
            v.wait_ge(dma_in, 16)
            # L1
            v.scalar_tensor_tensor(
                out=q01[:, 0:NB], in0=X[:, C_DS2:C_DS2 + NB], scalar=tS,
                in1=X[:, C_ABC:C_ABC + NB],
                op0=ALU.is_le, op1=ALU.mult, accum_out=D[:, 0:1])
            v.scalar_tensor_tensor(
                out=q01[:, NB:2 * NB], in0=X[:, C_DS2 + NB:C_DS2 + 2 * NB],
                scalar=tS, in1=X[:, C_ABC + NB:C_ABC + 2 * NB],
                op0=ALU.is_le, op1=ALU.mult, accum_out=D[:, 1:2])
            v.tensor_sub(out=td[:], in0=X[:, C_DSP:C_DSP + 6],
                         in1=X[:, C_DSQ:C_DSQ + 6])
            v.tensor_scalar(out=O[:, 0:1], in0=tS, scalar1=-b0,
                            scalar2=lp0c, op0=ALU.mult, op1=ALU.add)
            v.drain()
            # L2
            v.tensor_mul(out=mm[:], in0=q01[:], in1=X[:, C_DS2:C_DS2 + 2 * NB])
            v.tensor_add(out=dsh[:], in0=D[:], in1=X[:, C_EC:C_EC + 2])
            v.tensor_scalar(out=s0[:], in0=td[:], scalar1=_f32(TOL),
                            scalar2=dr0T, op0=ALU.is_gt, op1=ALU.mult)
            v.tensor_scalar(out=s2[:], in0=td[:], scalar1=_f32(-TOL),
                            scalar2=dr2T, op0=ALU.is_lt, op1=ALU.mult)
            if need_boundary:
                v.tensor_scalar(out=sbp[:], in0=td[:], scalar1=_f32(TOL),
                                scalar2=drbT, op0=ALU.is_equal, op1=ALU.mult)
                v.tensor_scalar(out=sbn[:], in0=td[:], scalar1=_f32(-TOL),
                                scalar2=drbT, op0=ALU.is_equal, op1=ALU.mult)
            v.drain()
            # L3: |dsh| via (dsh * -1) max dsh  (AP scalar -1; stt imm*mult
            # is miscompiled by this walrus)
            v.tensor_reduce(
                out=Mb[:], in_=mm[:].rearrange("p (f j) -> p f j", j=NB),
                axis=mybir.AxisListType.X, op=ALU.max)
            v.scalar_tensor_tensor(
                out=rrf[:], in0=s0[:], scalar=r1T, in1=s2[:],
                op0=ALU.add, op1=ALU.add)
            v.scalar_tensor_tensor(
                out=ab[:], in0=dsh[:], scalar=X[:, C_M1:C_M1 + 1],
                in1=dsh[:], op0=ALU.mult, op1=ALU.max)
            if need_boundary:
                v.drain()
                v.tensor_add(out=rrf2[:], in0=rrf[:], in1=sbp[:])
                v.drain()
                v.tensor_add(out=rrf3[:], in0=rrf2[:], in1=sbn[:])
            v.drain().then_inc(v1, 1)

            # ---- ACT computes feat = exp(-|dsh|/sigma), em = exp(rrfT),
            #      e1 = exp(mbt - t) ----
            v.wait_ge(a1, 1)
            # L4
            if need_mask:
                v.tensor_mul(out=emm[:], in0=em[:], in1=X[:, C_MSK:C_MSK + 6])
                v.drain()
            v.tensor_mul(out=erm[:], in0=emm[:], in1=rrf_f[:])
            v.tensor_reduce(
                out=Q[:, 0:2], in_=emm[:].rearrange("p (f k) -> p f k", k=3),
                axis=mybir.AxisListType.X, op=ALU.add)
            v.tensor_scalar_add(out=Q[:, 2:4], in0=e1[:], scalar1=1.0)
            v.tensor_mul(out=featFW[:], in0=feat[:], in1=X[:, C_FWT:C_FWT + 2])
            v.drain()
            # L5
            v.tensor_reduce(
                out=num[:], in_=erm[:].rearrange("p (f k) -> p f k", k=3),
                axis=mybir.AxisListType.X, op=ALU.add)
            v.tensor_sub(out=Y0[:].bitcast(I32),
                         in0=X[:, C_MG:C_MG + 4].bitcast(I32),
                         in1=Q[:].bitcast(I32))
            v.drain()
            # Newton-Raphson 1/Q, 2 iterations (InstReciprocal is unusable:
            # asynchronous writeback)
            v.tensor_mul(out=T1[:], in0=Q[:], in1=Y0[:])
            v.drain()
            v.tensor_scalar(out=T2[:], in0=T1[:], scalar1=-1.0, scalar2=2.0,
                            op0=ALU.mult, op1=ALU.add)
            v.drain()
            v.tensor_mul(out=Y1[:], in0=Y0[:], in1=T2[:])
            v.drain()
            v.tensor_mul(out=T1b[:], in0=Q[:], in1=Y1[:])
            v.drain()
            v.tensor_scalar(out=T2b[:], in0=T1b[:], scalar1=-1.0, scalar2=2.0,
                            op0=ALU.mult, op1=ALU.add)
            v.drain()
            v.tensor_mul(out=Y2[:], in0=Y1[:], in1=T2b[:])
            v.drain()
            # L12: colT = num * (1/den);  fs2 = feat*fw*(-T) * sigm
            # (sigm = 1/(1+e1) = Y2[:,2:4])
            v.tensor_mul(out=colT[:], in0=num[:], in1=Y2[:, 0:2])
            v.tensor_mul(out=fs2[:], in0=featFW[:], in1=Y2[:, 2:4])
            v.tensor_mul(out=sm[:], in0=Y2[:, 2:4], in1=Mb[:])
            v.drain()
            colT_f = colT
            if need_sel:
                v.tensor_mul(out=col2[:], in0=colT[:],
                             in1=X[:, C_SEL:C_SEL + 2])
                v.drain()
                v.tensor_add(out=col3[:], in0=col2[:],
                             in1=X[:, C_ADD:C_ADD + 2])
                v.drain()
                colT_f = col3
            # L13
            v.tensor_mul(out=sg[:], in0=fs2[:], in1=colT_f[:])
            v.drain()
            # L14
            v.tensor_scalar_add(out=cur[:], in0=sg[:], scalar1=b0)
            v.tensor_mul(out=term[:], in0=sm[:], in1=sg[:])
            v.drain()
            # L15: cur2 = cur*pi (folds +log(pi) into Ln); tcn = -t*cur
            v.tensor_mul(out=cur2[:], in0=cur[:], in1=X[:, C_PI:C_PI + 2])
            v.tensor_scalar(out=tcn[:], in0=cur[:], scalar1=tS, scalar2=-1.0,
                            op0=ALU.mult, op1=ALU.mult)
            v.drain().then_inc(v2, 1)
            # L16 (overlaps ACT Ln)
            v.tensor_add(out=ttx[:], in0=tcn[:], in1=term[:])
            v.drain()
            # ---- ACT computed lcur = Ln(cur*pi) ----
            v.wait_ge(a2, 1)
            v.tensor_add(out=O[:, 1:3], in0=lcur[:], in1=ttx[:])
            v.drain().then_inc(cdone, 1)

        @block.scalar
        def _(scalar):
            s = nc.scalar
            # preload the Exp activation table while the input DMA flies
            s.activation(de_o[:], dum_in, ACTF.Exp)
            s.wait_ge(v1, 1)
            s.activation(feat[:], ab[:], ACTF.Exp, scale=neg_inv_sigma)
            s.activation(em[:], rrf_f[:], ACTF.Exp)
            s.activation(e1[:], Mb[:], ACTF.Exp, bias=X[:, C_NT:C_NT + 1])
            s.drain().then_inc(a1, 1)
            # preload the Ln table while DVE runs the Newton division
            s.activation(dl_o[:], dum_in, ACTF.Ln)
            s.wait_ge(v2, 1)
            s.activation(lcur[:], cur2[:], ACTF.Ln)
            s.drain().then_inc(a2, 1)

    nc.finalize()
    return nc, ctx


def _prepare(t, data_sample, pi, A, base, formula_weight, prob):
    """Host-side bookkeeping + packed per-core inputs.  Returns (cfg, X)
    where X is [NCORES, P, NCOL] float32."""
    t = np.asarray(t, np.float32)
    ds = np.asarray(data_sample, np.float32)
    pi = np.asarray(pi, np.float32)
    A = np.asarray(A, np.float32)
    base = np.asarray(base, np.float32)
    fw = np.asarray(formula_weight, np.float32)
    prob = np.asarray(prob, np.float32)

    B = t.shape[0]
    P = -(-B // NCORES)  # rows per core (ceil)
    nF = A.shape[0]
    assert nF == 2 and ds.shape[1] == NB and A.shape[1] == NB + 2

    # --- A top-k bookkeeping (replicated, tiny) ---
    p_all = np.zeros(6, np.int64)
    q_all = np.zeros(6, np.int64)
    pv = np.zeros(6, np.float32)
    sel = np.zeros(2, np.float32)
    for i in range(nF):
        # top-3 by value desc, ties -> lower index first (lax.top_k semantics)
        idx = np.argsort(-A[i], kind="stable")[:KSEL]
        idx = np.sort(idx)
        valid = idx < NB
        pvi = (valid[_PA] & valid[_PB]).astype(np.float32)
        pv[3 * i:3 * i + 3] = pvi
        p_all[3 * i:3 * i + 3] = np.minimum(idx[_PA], NB - 1)
        q_all[3 * i:3 * i + 3] = np.minimum(idx[_PB], NB - 1)
        sel[i] = 1.0 if pvi.sum() > 0 else 0.0

    need_sel = bool((sel == 0.0).any())
    if need_sel:
        # keep den>0 so col is finite junk before the select overrides it
        for i in range(nF):
            if sel[i] == 0.0:
                pv[3 * i] = 1.0
    need_mask = bool((pv == 0.0).any())

    # --- piecewise-constant temporal-relation softmax values ---
    R0 = _rrf_region_value(0, prob)
    R1 = _rrf_region_value(1, prob)
    R2 = _rrf_region_value(2, prob)
    Rb = _rrf_region_value(-1, prob)

    dsP = ds[:, p_all]
    dsQ = ds[:, q_all]
    td_host = dsP - dsQ  # exactly what the device computes in f32
    need_boundary = bool((np.abs(td_host) == np.float32(TOL)).any())

    b0 = float(base[0])
    lp0c = _f32(np.float32(np.log(base[0])) + np.float32(np.log(pi[0])))

    # pre-scale by -1/TEMP: device computes rrfT = -rrf/TEMP directly
    sT = -1.0 / TEMP
    cfg = (
        int(P), _f32(R1 * sT), _f32((R0 - np.float32(R1)) * sT),
        _f32((R2 - np.float32(R1)) * sT),
        need_boundary, _f32((Rb - np.float32(R1)) * sT), need_mask, need_sel,
        _f32(-1.0 / SIGMA), _f32(b0), lp0c,
    )

    # --- pack per-core inputs ---
    BP = NCORES * P
    Xf = np.empty((BP, NCOL), np.float32)
    # benign padding rows (t=1, ds=0.5) keep all math finite
    ds_p = np.full((BP, NB), 0.5, np.float32)
    ds_p[:B] = ds
    t_p = np.ones((BP, 1), np.float32)
    t_p[:B] = t
    Xf[:, C_DS2:C_DS2 + NB] = ds_p
    Xf[:, C_DS2 + NB:C_DS2 + 2 * NB] = ds_p
    Xf[:, C_DSP:C_DSP + 6] = ds_p[:, p_all]
    Xf[:, C_DSQ:C_DSQ + 6] = ds_p[:, q_all]
    Xf[:, C_T:C_T + 1] = t_p
    arow = np.concatenate([A[0, :NB], A[1, :NB]])
    Xf[:, C_ABC:C_ABC + 2 * NB] = arow[None, :]
    ec = np.array([A[i, NB] + A[i, NB + 1] for i in range(nF)], np.float32) \
        - np.float32(KSEL)
    Xf[:, C_EC:C_EC + 2] = ec[None, :]
    Xf[:, C_FWT:C_FWT + 2] = (fw * np.float32(-TEMP))[None, :]
    Xf[:, C_PI:C_PI + 2] = pi[1:][None, :]
    Xf[:, C_NT:C_NT + 1] = -t_p
    Xf[:, C_M1:C_M1 + 1] = -1.0
    Xf[:, C_MG:C_MG + 4] = np.full((1, 4), MAGIC, np.int32).view(np.float32)
    Xf[:, C_MSK:C_MSK + 6] = pv[None, :]
    Xf[:, C_SEL:C_SEL + 2] = sel[None, :]
    Xf[:, C_ADD:C_ADD + 2] = ((1.0 - sel) * np.float32(-1.0 / TEMP))[None, :]

    return cfg, Xf.reshape(NCORES, P, NCOL)


def kernel(t, data_sample, pi, A, base, formula_weight, prob):
    global LAST_RESULT
    cfg, X = _prepare(t, data_sample, pi, A, base, formula_weight, prob)
    B = np.asarray(t).shape[0]
    P = cfg[0]

    cached = _BUILD_CACHE.get(cfg)
    if cached is None:
        cached = _build(cfg)
        _BUILD_CACHE[cfg] = cached
    nc, _ctx = cached

    in_maps = [{"x": np.ascontiguousarray(X[c])} for c in range(NCORES)]
    res = run_bass_kernel_spmd(nc, in_maps, core_ids=list(range(NCORES)))
    LAST_RESULT = res
    out = np.concatenate([res.results[c]["o"] for c in range(NCORES)], axis=0)
    return np.ascontiguousarray(out[:B]).astype(np.float32)
